# revision 63
# baseline (speedup 1.0000x reference)
"""Chamfer distance loss (truncated, non-squared) on 8 Trainium2 NeuronCores.

Problem: src_points (2,16384,3) f32, tgt_points (2,16384,3) f32 ->
scalar loss = masked_mean(src_nn) + masked_mean(tgt_nn), where
src_nn[b,n] = min_m dist(src[b,n], tgt[b,m]) (safe sqrt, eps=1e-12),
mask = dist < 0.5, masked mean over all B*N elements (count clamped >= 1).

Sharding: rows of src_points (N=16384) are split across 8 cores (2048 each).
Each core computes its (2048 x 16384) distance tile per batch:
  - row-min over tgt (exact per core) -> local masked sum/count scalars
  - column-min over its 2048 src rows -> partial tgt_nn, combined across
    cores with one on-device AllReduce(min); the per-core src-side scalars
    ride through the same AllReduce in +inf-padded per-core slots.
Core 0's output (identical on all cores) is the final scalar loss.

Math on device: squared distances come from a single fp16-split matmul
(x = xh + xl in fp16; products xh*yh + xl*yh + xh*yl accumulate in fp32
PSUM - the ~1e-5 xl*yl term is dropped; ||y||^2 enters as three fp16 rhs
rows against ones; ||x||^2 is added per-partition by the ScalarE bias while
staging PSUM->SBUF as fp16). This gives ~fp32 accuracy at the full fp16 PE
rate. Minima are taken over squared distances (monotone), and the
sqrt/mask/mean run on tiny final vectors.

Schedule (the DVE is the bottleneck engine - it runs both min streams at
the fp16 2x rate and is ~95% occupied in steady state; every choice below
cuts DVE element touches or pipeline stalls):
 - m-chunks are processed in PAIRS: both ScalarE PSUM->SBUF stages of a
   pair land in one [128, 4096] fp16 buffer, so each column-side running
   min is ONE fp16 2x-rate tensor_tensor over free-size 4096.
 - the s=0 sweep's ScalarE stages write straight into colacc (s=0
   distances ARE the initial column minima), removing all eight 4096-wide
   DVE copies per batch; the s=0 row ops read the colacc slices.
 - row side per s: the first tensor_tensor consumes TWO fresh pair-buffers
   (both operands fresh data, 2 elements/cycle/lane), two more fold the
   remaining pairs, one half-width fold then a single 1x tensor_reduce.
 - the column-min cross-partition reduce transposes colacc 128x128 blocks
   on the TensorE (identity-matmul into fp16 PSUM) and min-reduces straight
   from PSUM; the DMA-xbar transposes it replaces serialized ~50us of
   SP-queue work per batch into the next batch's prep DMAs.
 - ALL operand prep (fp16 point splits, the exact -2*fp16 scaling, the
   ||y||^2 3-way split, ones rows, row duplication) happens on the host in
   make_in_maps: lhsT and rhs arrive in their exact on-chip layouts and
   stream in as contiguous DMA loads (rhs per 4096-column pair, interleaved
   with the s=0 sweep), so no engine spends cycles on prep.
Known toolchain limits (verified by HW bisect, exact-in-simulator): the
DVE tensor_tensor_reduce instruction hangs the device, gpsimd elementwise
compute ops (tensor_tensor/tensor_scalar/tensor_copy) fail NEFF lowering,
and SWDGE partition-gather DMAs cost ~66ns/descriptor of Pool-engine
descriptor generation (making on-device [128,q,C]->[C,m] reshuffles slower
than host marshalling). tensor_tensor_reduce alone would cut the DVE floor
by another ~7%.
"""

import os

import numpy as np

# recover cleanly if a previous crashed run left the NeuronCores wedged
os.environ.setdefault("NEURON_RT_RESET_CORES", "1")

import concourse.bass as bass
import concourse.bacc as bacc
import concourse.mybir as mybir
import concourse.tile as tile
from concourse import bass2jax

F32 = mybir.dt.float32
F16 = mybir.dt.float16
AF = mybir.ActivationFunctionType
ALU = mybir.AluOpType
AX = mybir.AxisListType

N_CORES = 8
B = 2
N = 16384          # src points per batch (full)
M = 16384          # tgt points per batch
C = 3
K = 12             # matmul contraction rows: xh*yh, xl*yh, xh*yl, + 3 ny rows
                   # (the xl*yl cross term is ~1e-5 absolute and dropped)
TRUNC = 0.5
EPS = 1e-12
BIG = 1.0e30


def build_program(n_cores=N_CORES, n=N, m=M, b_sz=B, debug_outs=False, repeat=1,
                  rhs_bufs=2, colacc_bufs=1, d_bufs=3, prep_y_bufs=1, rowacc_bufs=2,
                  tr_bufs=2, pool_rowacc_every=0, pool_colacc_every=0,
                  collective=True, use_petr=True, use_gp_ny=True):
    nsh = n // n_cores          # src rows per core per batch
    s_tiles = nsh // 128        # src tiles of 128 partitions
    m_super = 2048              # tgt columns per supertile (4 PSUM banks)
    m_tiles = m // m_super
    banks = m_super // 512
    n_slots = 2 * n_cores
    cc_len = b_sz * m + n_slots

    nc = bacc.Bacc(
        "TRN2",
        target_bir_lowering=False,
        debug=False,
        num_devices=n_cores,
    )

    # host-marshalled operands: lhsT/rhs arrive in their exact on-chip
    # layouts (fp16 splits, -2 scaling, ones rows, ny rows all precomputed
    # in make_in_maps), so the device does zero prep compute
    lhsT_d = nc.dram_tensor("lhsT_d", [b_sz, K, nsh], F16, kind="ExternalInput")
    nx_d = nc.dram_tensor("nx_d", [b_sz, 128, s_tiles], F32, kind="ExternalInput")
    rhs_d = nc.dram_tensor("rhs_d", [b_sz, K, m], F16, kind="ExternalInput")
    inf_mask = nc.dram_tensor("inf_mask", [1, n_slots], F32, kind="ExternalInput")
    ident_d = nc.dram_tensor("ident", [128, 128], F16, kind="ExternalInput")
    loss_out = nc.dram_tensor("loss_out", [1, 1], F32, kind="ExternalOutput")

    cc_in = nc.dram_tensor("cc_in", [cc_len], F32)
    cc_out = nc.dram_tensor("cc_out", [cc_len], F32)

    if debug_outs:
        dbg_rowfin = nc.dram_tensor(
            "dbg_rowfin", [128, b_sz * s_tiles], F32, kind="ExternalOutput"
        )
        dbg_colfin0 = nc.dram_tensor(
            "dbg_colfin0", [128, m // 128], F32, kind="ExternalOutput"
        )
        dbg_slots = nc.dram_tensor("dbg_slots", [1, n_slots], F32, kind="ExternalOutput")
        dbg_gslots = nc.dram_tensor(
            "dbg_gslots", [1, n_slots], F32, kind="ExternalOutput"
        )
        dbg_d0 = nc.dram_tensor("dbg_d0", [128, m_super], F32, kind="ExternalOutput")
        dbg_nx = nc.dram_tensor("dbg_nx", [128, s_tiles], F32, kind="ExternalOutput")
        dbg_tpair = nc.dram_tensor("dbg_tpair", [1, 2], F32, kind="ExternalOutput")
        dbg_spair = nc.dram_tensor("dbg_spair", [1, 2], F32, kind="ExternalOutput")

    with tile.TileContext(nc) as tc:
        with (
            tc.tile_pool(name="lhs", bufs=2) as lhs_pool,
            tc.tile_pool(name="rhs", bufs=rhs_bufs) as rhs_pool,
            tc.tile_pool(name="colacc", bufs=colacc_bufs) as colacc_pool,
            tc.tile_pool(name="prep_x", bufs=1) as prep_x_pool,
            tc.tile_pool(name="prep_y", bufs=prep_y_bufs) as prep_y_pool,
            tc.tile_pool(name="norm", bufs=2) as norm_pool,
            tc.tile_pool(name="d", bufs=d_bufs) as d_pool,
            tc.tile_pool(name="rowacc", bufs=rowacc_bufs) as rowacc_pool,
            tc.tile_pool(name="tr", bufs=tr_bufs) as tr_pool,
            tc.tile_pool(name="fin", bufs=1) as fin_pool,
            tc.tile_pool(name="psum", bufs=2, space="PSUM") as psum_pool,
        ):
            # persistent result tiles
            rowfin = fin_pool.tile([128, b_sz * s_tiles], F32, tag="rowfin")
            colfin = [
                fin_pool.tile(
                    [128, m // 128], F32, name=f"colfin{b}", tag=f"colfin{b}"
                )
                for b in range(b_sz)
            ]

            # identity for PE-based 128x128 transposes (column-min reduce)
            ident = fin_pool.tile([128, 128], F16, tag="ident")
            nc.sync.dma_start(ident[:], ident_d[:])
            # initializing write so the allocator sees rowfin before the
            # tensor_tensor_reduce accum writes (only) land in it
            nc.vector.memset(rowfin[:], 0.0)

            for b in [bb for _ in range(repeat) for bb in range(b_sz)]:
                # ---- per-batch prep: pure DMA loads, no compute ----
                lhsT = lhs_pool.tile([K, nsh], F16, tag="lhsT")
                nc.sync.dma_start(lhsT[:], lhsT_d[b])
                nx = norm_pool.tile([128, s_tiles], F32, tag="nx")
                nc.sync.dma_start(nx[:], nx_d[b])
                if debug_outs and b == 0:
                    nc.sync.dma_start(dbg_nx[:], nx[:])

                # ---- rhs [K, m] f16, loaded per 4096-column pair ----
                rhs = rhs_pool.tile([K, m], F16, tag="rhs")

                # ---- main loop ----
                # The s=0 sweep is interleaved with the per-chunk rhs prep so
                # each engine's FIFO alternates prep and compute work instead
                # of draining all prep first (saves the cold-start stall).
                #
                # Tiles are processed in PAIRS of m-chunks. Both halves of a
                # pair land in one [128, 2*m_super] fp16 buffer so that:
                #  - the column-side running min is ONE tensor_tensor(min) of
                #    free-size 2*m_super (amortizes the per-op DVE overhead),
                #  - the row side is ONE tensor_tensor_reduce that consumes
                #    BOTH fresh halves (in0/in1) and chains the row-min through
                #    its accumulator (scalar -> accum_out), replacing a
                #    per-tile min-accumulate chain plus a separate 1x-rate
                #    tensor_reduce per s.
                colacc = colacc_pool.tile([128, m], F16, tag="colacc")
                m_pairs = m_tiles // 2

                def prep_pair(pj):
                    sl = slice(pj * 2 * m_super, (pj + 1) * 2 * m_super)
                    nc.sync.dma_start(rhs[:, sl], rhs_d[b, :, sl])

                def emit_pair(s, pj):
                    psl = slice(pj * 2 * m_super, (pj + 1) * 2 * m_super)
                    if s == 0:
                        # s=0 distances ARE the initial colacc: ScalarE writes
                        # them straight into colacc, skipping the DVE copy
                        prep_pair(pj)
                        dest, dbase = colacc, pj * 2 * m_super
                        dd = None
                    else:
                        dd = d_pool.tile([128, 2 * m_super], F16, tag="dd")
                        dest, dbase = dd, 0
                    for h in range(2):
                        mi = 2 * pj + h
                        psum = psum_pool.tile([128, m_super], F32, tag="psum")
                        for j in range(banks):
                            nc.tensor.matmul(
                                psum[:, j * 512 : (j + 1) * 512],
                                lhsT[:, s * 128 : (s + 1) * 128],
                                rhs[
                                    :,
                                    mi * m_super
                                    + j * 512 : mi * m_super
                                    + (j + 1) * 512,
                                ],
                                start=True,
                                stop=True,
                            )
                        nc.scalar.activation(
                            dest[:, dbase + h * m_super : dbase + (h + 1) * m_super],
                            psum[:],
                            AF.Identity,
                            bias=nx[:, s : s + 1],
                        )
                    if s == 0:
                        return colacc[:, psl]
                    # optionally hand a fixed set of column-pair ranges to the
                    # gpsimd engine so its col-min chains never hop engines
                    on_pool = pool_colacc_every > 0 and (
                        pj % pool_colacc_every == pool_colacc_every - 1
                    )
                    eng = nc.gpsimd if on_pool else nc.vector
                    eng.tensor_tensor(colacc[:, psl], colacc[:, psl], dd[:], ALU.min)
                    return dd[:]

                # Row side: the first TT consumes TWO fresh pair-buffers at
                # the fp16 2x rate, later TTs fold further pairs into the
                # accumulator; a fold to [128, m_super] then one 1x reduce
                # finishes the row.
                for s in range(s_tiles):
                    rowacc4 = rowacc_pool.tile(
                        [128, 2 * m_super], F16, tag="rowacc4", bufs=1
                    )
                    dd_prev = None
                    for pj in range(m_pairs):
                        dd = emit_pair(s, pj)
                        if pj == 0:
                            dd_prev = dd
                        elif pj == 1:
                            nc.vector.tensor_tensor(
                                rowacc4[:], dd_prev, dd, ALU.min
                            )
                        else:
                            nc.vector.tensor_tensor(
                                rowacc4[:], rowacc4[:], dd, ALU.min
                            )
                    # halve at the 2x TT rate down to 256 before paying the
                    # 1x-rate tensor_reduce (saves ~0.8us per s-block)
                    rowacc2 = rowacc_pool.tile(
                        [128, m_super], F16, tag="rowacc2", bufs=1
                    )
                    nc.vector.tensor_tensor(
                        rowacc2[:],
                        rowacc4[:, 0:m_super],
                        rowacc4[:, m_super : 2 * m_super],
                        ALU.min,
                    )
                    w = m_super // 2
                    while w >= 256:
                        nc.vector.tensor_tensor(
                            rowacc2[:, 0:w],
                            rowacc2[:, 0:w],
                            rowacc2[:, w : 2 * w],
                            ALU.min,
                        )
                        w //= 2
                    nc.vector.tensor_reduce(
                        rowfin[:, b * s_tiles + s : b * s_tiles + s + 1],
                        rowacc2[:, 0 : 2 * w],
                        axis=AX.X,
                        op=ALU.min,
                    )

                # ---- column-min partition reduce via PE transpose ----
                # TensorE transposes 128x128 blocks of colacc into PSUM (fp16
                # out, 2 banks per [128, m_super] group); DVE min-reduces each
                # group straight from PSUM. Avoids the 128 DMA-xbar transposes
                # whose SP-queue head-of-line blocking stalled each batch
                # boundary for ~50us.
                # fp16 [128, 2*m_super] PSUM tiles occupy the same 4-bank slot
                # as the matmul fp32 [128, m_super] tiles, so they share the
                # pool tag; 32 transposed blocks amortize one 1x-rate reduce.
                if use_petr:
                    nj2 = 2 * m_super // 128
                    for mp in range(m_tiles // 2):
                        tp = psum_pool.tile([128, 2 * m_super], F16, tag="psum")
                        for j in range(nj2):
                            off = mp * 2 * m_super + j * 128
                            nc.tensor.transpose(
                                tp[:, j * 128 : (j + 1) * 128],
                                colacc[:, off : off + 128],
                                ident[:],
                            )
                        nc.vector.tensor_reduce(
                            colfin[b][:, mp * nj2 : (mp + 1) * nj2],
                            tp.rearrange("p (j q) -> p j q", q=128),
                            axis=AX.X,
                            op=ALU.min,
                        )
                else:
                    nj = m_super // 128
                    for mi in range(m_tiles):
                        tr = tr_pool.tile([128, m_super], F16, tag="trd")
                        for j in range(nj):
                            off = mi * m_super + j * 128
                            nc.sync.dma_start_transpose(
                                tr[:, j * 128 : (j + 1) * 128],
                                colacc[:, off : off + 128],
                            )
                        nc.vector.tensor_reduce(
                            colfin[b][:, mi * nj : (mi + 1) * nj],
                            tr.rearrange("p (j q) -> p j q", q=128),
                            axis=AX.X,
                            op=ALU.min,
                        )

            # ---- src-side local stats ----
            st = fin_pool.tile([128, b_sz * s_tiles], F32, tag="st")
            nc.vector.tensor_scalar(st[:], rowfin[:], EPS, None, op0=ALU.max)
            sdist = fin_pool.tile([128, b_sz * s_tiles], F32, tag="sdist")
            nc.scalar.activation(sdist[:], st[:], AF.Sqrt)
            smask = fin_pool.tile([128, b_sz * s_tiles], F32, tag="smask")
            nc.vector.tensor_scalar(smask[:], sdist[:], TRUNC, None, op0=ALU.is_lt)
            smd = fin_pool.tile([128, b_sz * s_tiles], F32, tag="smd")
            nc.vector.tensor_tensor(smd[:], sdist[:], smask[:], ALU.mult)
            spair = fin_pool.tile([128, 2], F32, tag="spair")
            nc.vector.tensor_reduce(spair[:, 0:1], smd[:], axis=AX.X, op=ALU.add)
            nc.vector.tensor_reduce(spair[:, 1:2], smask[:], axis=AX.X, op=ALU.add)
            ones = fin_pool.tile([128, 1], F32, tag="ones")
            nc.vector.memset(ones[:], 1.0)
            ssum_ps = psum_pool.tile([1, 2], F32, tag="psum")
            nc.tensor.matmul(ssum_ps[:], ones[:], spair[:], start=True, stop=True)

            # slots = broadcast(ssum_ps) + inf_mask  (only own slots finite)
            imask = fin_pool.tile([1, n_slots], F32, tag="imask")
            nc.sync.dma_start(imask[:], inf_mask[:])
            slots = fin_pool.tile([1, n_slots], F32, tag="slots")
            nc.vector.tensor_tensor(
                slots[:],
                ssum_ps
                .rearrange("p (o t) -> p o t", o=1)
                .to_broadcast([1, n_slots // 2, 2]),
                imask.rearrange("p (o t) -> p o t", t=2),
                ALU.add,
            )

            if debug_outs:
                nc.sync.dma_start(dbg_rowfin[:], rowfin[:])
                nc.sync.dma_start(dbg_colfin0[:], colfin[0][:])
                nc.sync.dma_start(dbg_slots[:], slots[:])

            # ---- pack + AllReduce(min) ----
            for b in range(b_sz):
                nc.gpsimd.dma_start(
                    cc_in[b * m : (b + 1) * m].rearrange("(p q) -> p q", p=128),
                    colfin[b][:],
                )
            nc.gpsimd.dma_start(
                cc_in[b_sz * m : cc_len].rearrange("(o q) -> o q", o=1),
                slots[0:1, :],
            )
            if collective:
                nc.gpsimd.collective_compute(
                    "AllReduce",
                    ALU.min,
                    replica_groups=[list(range(n_cores))],
                    ins=[cc_in.ap()],
                    outs=[cc_out.ap()],
                )
            else:
                # sim-only stand-in so single-core CoreSim doesn't block
                nc.gpsimd.dma_start(
                    cc_out.rearrange("(o q) -> o q", o=1),
                    cc_in.rearrange("(o q) -> o q", o=1),
                )

            # ---- tgt-side stats on globally reduced mins ----
            gt = fin_pool.tile([128, b_sz * m // 128], F32, tag="gt")
            nc.gpsimd.dma_start(
                gt[:], cc_out[0 : b_sz * m].rearrange("(p q) -> p q", p=128)
            )
            gslots = fin_pool.tile([1, n_slots], F32, tag="gslots")
            nc.gpsimd.dma_start(
                gslots[:], cc_out[b_sz * m : cc_len].rearrange("(o q) -> o q", o=1)
            )

            nc.vector.tensor_scalar(gt[:], gt[:], EPS, None, op0=ALU.max)
            gtd = fin_pool.tile([128, b_sz * m // 128], F32, tag="gtd")
            nc.scalar.activation(gtd[:], gt[:], AF.Sqrt)
            gtm = fin_pool.tile([128, b_sz * m // 128], F32, tag="gtm")
            nc.vector.tensor_scalar(gtm[:], gtd[:], TRUNC, None, op0=ALU.is_lt)
            tpair = fin_pool.tile([128, 2], F32, tag="tpair")
            nc.vector.tensor_reduce(tpair[:, 1:2], gtm[:], axis=AX.X, op=ALU.add)
            nc.vector.tensor_tensor(gtm[:], gtd[:], gtm[:], ALU.mult)
            nc.vector.tensor_reduce(tpair[:, 0:1], gtm[:], axis=AX.X, op=ALU.add)
            tsum_ps = psum_pool.tile([1, 2], F32, tag="psum")
            nc.tensor.matmul(tsum_ps[:], ones[:], tpair[:], start=True, stop=True)

            if debug_outs:
                nc.sync.dma_start(dbg_gslots[:], gslots[:])
                tpair_dbg = fin_pool.tile([1, 2], F32, tag="tpair_dbg")
                nc.vector.tensor_copy(tpair_dbg[:], tsum_ps[:])
                nc.sync.dma_start(dbg_tpair[:], tpair_dbg[:])

            # src global: sum the per-core (sum, cnt) slot pairs
            spair_g = fin_pool.tile([1, 2], F32, tag="spair_g")
            nc.vector.tensor_reduce(
                spair_g[:],
                gslots.rearrange("p (c t) -> p t c", t=2),
                axis=AX.X,
                op=ALU.add,
            )
            if debug_outs:
                nc.sync.dma_start(dbg_spair[:], spair_g[:])

            # loss = s_sum/max(s_cnt,1) + t_sum/max(t_cnt,1)
            sums = fin_pool.tile([1, 2], F32, tag="sums")
            nc.vector.tensor_copy(sums[:, 0:1], spair_g[:, 0:1])
            nc.vector.tensor_copy(sums[:, 1:2], tsum_ps[:, 0:1])
            cnts = fin_pool.tile([1, 2], F32, tag="cnts")
            nc.vector.tensor_copy(cnts[:, 0:1], spair_g[:, 1:2])
            nc.vector.tensor_copy(cnts[:, 1:2], tsum_ps[:, 1:2])
            cnts2 = fin_pool.tile([1, 2], F32, tag="cnts2")
            nc.vector.tensor_scalar(cnts2[:], cnts[:], 1.0, None, op0=ALU.max)
            rec = fin_pool.tile([1, 2], F32, tag="rec")
            nc.vector.reciprocal(rec[:], cnts2[:])
            terms = fin_pool.tile([1, 2], F32, tag="terms")
            nc.vector.tensor_tensor(terms[:], sums[:], rec[:], ALU.mult)
            lossv = fin_pool.tile([1, 1], F32, tag="lossv")
            nc.vector.tensor_reduce(lossv[:], terms[:], axis=AX.X, op=ALU.add)
            nc.sync.dma_start(loss_out[:, :], lossv[:])

    nc.compile()
    return nc


def make_in_maps(src, tgt, n_cores=N_CORES):
    """Host-side input marshalling: fp16 point splits, the -2 scaling, the
    ||y||^2 3-way split, and the exact on-chip lhsT/rhs row layouts are all
    precomputed here so the device spends no compute on prep."""
    src = np.ascontiguousarray(src, dtype=np.float32)
    tgt = np.ascontiguousarray(tgt, dtype=np.float32)
    b_sz, n, _ = src.shape
    m = tgt.shape[1]
    nsh = n // n_cores
    s_tiles = nsh // 128
    n_slots = 2 * n_cores

    # rhs rows (shared by all cores): yh, yh, yl, [nyh, nym, nyl]
    yh = tgt.astype(np.float16)
    yl = (tgt - yh.astype(np.float32)).astype(np.float16)
    ny = np.einsum("bmc,bmc->bm", tgt, tgt).astype(np.float32)
    nyh = ny.astype(np.float16)
    rem = ny - nyh.astype(np.float32)
    nym = rem.astype(np.float16)
    nyl = (rem - nym.astype(np.float32)).astype(np.float16)
    yh_t = yh.transpose(0, 2, 1)
    rhs_d = np.ascontiguousarray(
        np.concatenate(
            [yh_t, yh_t, yl.transpose(0, 2, 1), nyh[:, None], nym[:, None],
             nyl[:, None]],
            axis=1,
        )
    )
    assert rhs_d.shape == (b_sz, K, m)

    in_maps = []
    for c in range(n_cores):
        xs = src[:, c * nsh : (c + 1) * nsh, :]
        # lhsT rows: -2*xh, -2*xl, -2*xh, ones (all exact fp16 ops)
        xh = xs.astype(np.float16)
        xl = (xs - xh.astype(np.float32)).astype(np.float16)
        m2xh = (np.float16(-2) * xh).transpose(0, 2, 1)
        m2xl = (np.float16(-2) * xl).transpose(0, 2, 1)
        ones = np.ones((b_sz, K - 3 * C, nsh), dtype=np.float16)
        lhsT_d = np.ascontiguousarray(
            np.concatenate([m2xh, m2xl, m2xh, ones], axis=1)
        )
        # nx[b, p, s] = |x_{s*128+p}|^2
        nx = np.einsum("bnc,bnc->bn", xs, xs).astype(np.float32)
        nx_d = np.ascontiguousarray(
            nx.reshape(b_sz, s_tiles, 128).transpose(0, 2, 1)
        )
        imask = np.full((1, n_slots), BIG, dtype=np.float32)
        imask[0, 2 * c] = 0.0
        imask[0, 2 * c + 1] = 0.0
        in_maps.append(
            {
                "lhsT_d": lhsT_d,
                "nx_d": nx_d,
                "rhs_d": rhs_d,
                "inf_mask": imask,
                "ident": np.eye(128, dtype=np.float16),
            }
        )
    return in_maps


def make_runner(nc, n_cores=N_CORES):
    """Build a reusable callable (in_maps) -> per-core output dicts.

    Same lowering as bass2jax.run_bass_via_pjrt, but the jitted shard_map
    callable is constructed once and reused, so repeat calls skip retracing.
    """
    import jax
    import jax.numpy as jnp
    from jax.sharding import Mesh, PartitionSpec
    from jax.experimental.shard_map import shard_map
    import concourse.mybir as _mybir

    bass2jax.install_neuronx_cc_hook()
    from concourse.bass2jax import _bass_exec_p, partition_id_tensor

    partition_name = nc.partition_id_tensor.name if nc.partition_id_tensor else None
    in_names, out_names, out_avals, zero_outs = [], [], [], []
    for alloc in nc.m.functions[0].allocations:
        if not isinstance(alloc, _mybir.MemoryLocationSet):
            continue
        name = alloc.memorylocations[0].name
        if alloc.kind == "ExternalInput":
            if name != partition_name:
                in_names.append(name)
        elif alloc.kind == "ExternalOutput":
            out_names.append(name)
            shape = tuple(alloc.tensor_shape)
            dtype = _mybir.dt.np(alloc.dtype)
            out_avals.append(jax.core.ShapedArray(shape, dtype))
            zero_outs.append(np.zeros(shape, dtype))
    n_params = len(in_names)
    n_outs = len(out_avals)
    all_in_names = list(in_names) + list(out_names)
    if partition_name is not None:
        all_in_names.append(partition_name)
    donate = tuple(range(n_params, n_params + n_outs))

    def _body(*args):
        operands = list(args)
        if partition_name is not None:
            operands.append(partition_id_tensor())
        outs = _bass_exec_p.bind(
            *operands,
            out_avals=tuple(out_avals),
            in_names=tuple(all_in_names),
            out_names=tuple(out_names),
            lowering_input_output_aliases=(),
            sim_require_finite=True,
            sim_require_nnan=True,
            nc=nc,
        )
        return tuple(outs)

    devices = jax.devices()[:n_cores]
    mesh = Mesh(np.asarray(devices), ("core",))
    in_specs = (PartitionSpec("core"),) * (n_params + n_outs)
    out_specs = (PartitionSpec("core"),) * n_outs
    sharded = jax.jit(
        shard_map(
            _body, mesh=mesh, in_specs=in_specs, out_specs=out_specs, check_rep=False
        ),
        donate_argnums=donate,
        keep_unused=True,
    )

    from jax.sharding import NamedSharding

    in_sharding = NamedSharding(mesh, PartitionSpec("core"))

    def prepare(in_maps):
        concat_in = [
            np.concatenate([np.asarray(in_maps[c][nm]) for c in range(n_cores)], axis=0)
            for nm in in_names
        ]
        return [jax.device_put(a, in_sharding) for a in concat_in]

    def run_prepared(prepared, block=False):
        concat_zeros = [
            np.zeros((n_cores * z.shape[0], *z.shape[1:]), z.dtype) for z in zero_outs
        ]
        out_arrs = sharded(*prepared, *concat_zeros)
        if block:
            for o in out_arrs:
                o.block_until_ready()
        return out_arrs

    def run(in_maps):
        out_arrs = run_prepared(prepare(in_maps))
        return [
            {
                nm: np.asarray(out_arrs[i]).reshape(n_cores, *out_avals[i].shape)[c]
                for i, nm in enumerate(out_names)
            }
            for c in range(n_cores)
        ]

    run.prepare = prepare
    run.run_prepared = run_prepared
    return run


_CACHE: dict = {}


def _get_runner():
    if "runner" not in _CACHE:
        nc = build_program()
        _CACHE["nc"] = nc
        _CACHE["runner"] = make_runner(nc)
    return _CACHE["runner"]


def kernel(src_points: np.ndarray, tgt_points: np.ndarray) -> np.ndarray:
    runner = _get_runner()
    in_maps = make_in_maps(np.asarray(src_points), np.asarray(tgt_points))
    results = runner(in_maps)
    loss = np.float32(results[0]["loss_out"][0, 0])
    return np.asarray(loss, dtype=np.float32).reshape(())

